# revision 9
# baseline (speedup 1.0000x reference)
"""Trainium2 Bass kernel for ParallelLMHeadWithLoRA.

out[t, v] = hidden[t] @ weight[v]^T + xa[t] @ lora_B[e_t, v]^T
            where xa[t] = hidden[t] @ lora_A[e_t]^T,  e_t = weight_indices[t]

Sharding: column-parallel on vocab across 8 cores — weight and lora_B are
sharded along V (4000 rows/core), hidden / lora_A / weight_indices are
replicated.  Each core computes out[:, shard]; the host concatenates.

Per-core schedule (all fp32 data, fp32r matmuls).  hT is a 1024-token
ring of transposed-hidden slots; token sweeps map token tiles onto hT
slots mod 1024 so the next sweep's hT build overlaps the current sweep's
matmuls (disjoint slots):
  - consts, lora_A^T -> ATs, lora_B^T -> BTs (DRAM scratches)
  - sweep I   (tokens    0:512,  slots 0:512,   4 PSUM banks): FUSED —
    per vocab panel and k-tile, PE-transpose the natural weight block,
    matmul immediately, and store the transposed panel to WTs scratch.
  - sweep II  (tokens 512:1280,  slots 512:1024 + 0:256, 6 banks): WTs
  - sweep III (tokens 1280:2048, slots 256:1024, 6 banks): WTs
"""

import numpy as np

T = 2048
H = 4096
V = 32000
NCORES = 8
VS = V // NCORES  # 4000
E = 8
R = 16
P = 128
KT = H // P  # 32
TB = 1024  # hT slot-ring size (tokens)
PANELS = [(i * 512, 512) for i in range(7)] + [(3584, VS - 3584)]  # 7x512 + 416

_CACHE = {}


def _build_nc():
    from concourse import bacc
    import concourse.mybir as mybir
    from concourse.tile import TileContext
    from concourse.masks import make_identity

    f32 = mybir.dt.float32
    f32r = mybir.dt.float32r
    bf16 = mybir.dt.bfloat16
    i32 = mybir.dt.int32
    OP = mybir.AluOpType

    nc = bacc.Bacc("TRN2", target_bir_lowering=False, debug=False)

    hid_d = nc.dram_tensor("hidden", [T, H], f32, kind="ExternalInput")
    w_d = nc.dram_tensor("weight", [VS, H], f32, kind="ExternalInput")
    la_d = nc.dram_tensor("lora_a", [P, H], f32, kind="ExternalInput")  # [E*R, H]
    lb_d = nc.dram_tensor("lora_b", [E, VS, R], f32, kind="ExternalInput")
    idx_d = nc.dram_tensor("widx", [1, T], i32, kind="ExternalInput")
    out_d = nc.dram_tensor("out", [T, VS], f32, kind="ExternalOutput")

    wts_d = nc.dram_tensor("wts", [KT, P, VS], f32r, kind="Internal")
    ats_d = nc.dram_tensor("ats", [KT, P, P], f32r, kind="Internal")
    bts_d = nc.dram_tensor("bts", [P, VS], f32r, kind="Internal")

    with TileContext(nc) as tc:
        ident, free_ident = tc.tile([P, P], f32, name="ident")
        make_identity(nc, ident)
        hT, free_hT = tc.tile([P, KT * TB], f32r, name="hT")
        hT_k = hT.rearrange("p (k t) -> p k t", t=TB)
        mxaT, _free_mxa = tc.tile([P, TB], f32r, name="mxaT")

        with (
            tc.tile_pool(name="psp", bufs=8, space="PSUM") as psp,
            tc.tile_pool(name="natp", bufs=5) as natp,      # [128,512] f32 staging
            tc.tile_pool(name="wstp", bufs=2) as wstp,      # [128,512] f32r wT tiles
            tc.tile_pool(name="nathp", bufs=3) as nathp,    # [128,1024] f32 staging
            tc.tile_pool(name="wldp", bufs=3) as wldp,      # [128,2048] f32r loads
            tc.tile_pool(name="atldp", bufs=2) as atldp,    # [128,512] f32r loads
            tc.tile_pool(name="btldp", bufs=1) as btldp,    # [128,512] f32r loads
            tc.tile_pool(name="ostp", bufs=2) as ostp,      # [128,512] f32 out staging
            tc.tile_pool(name="maskp", bufs=1) as maskp,
            tc.tile_pool(name="btstp", bufs=1) as btstp,    # [16,512] f32r staging
        ):
            # ---- constants for mask ----
            p_col_i = maskp.tile([P, 1], i32, tag="pci")
            nc.gpsimd.iota(p_col_i, pattern=[[0, 1]], base=0, channel_multiplier=1)
            p_col = maskp.tile([P, 1], f32, tag="pcf")
            nc.vector.tensor_copy(p_col, p_col_i)

            # ---- lora_A^T -> ATs ----
            for q in range(4):
                nat_a = nathp.tile([P, 1024], f32, tag="nath")
                nc.sync.dma_start(nat_a, la_d[:, q * 1024:(q + 1) * 1024])
                for ks in range(2):
                    ps = psp.tile([P, 512], f32, tag="bank")
                    for kk in range(4):
                        nc.tensor.transpose(
                            ps[:, kk * P:(kk + 1) * P],
                            nat_a[:, (ks * 4 + kk) * P:(ks * 4 + kk + 1) * P],
                            ident,
                        )
                    st = wstp.tile([P, 512], f32r, tag="wst")
                    nc.vector.tensor_copy(st, ps)
                    k0 = q * 8 + ks * 4
                    nc.sync.dma_start(
                        ats_d[k0:k0 + 4, :, :].rearrange("k h e -> h k e"), st
                    )

            # ---- lora_B^T -> BTs ----
            NB_FULL = VS // P  # 31
            REM = VS % P       # 32
            for e in range(E):
                nat_b = natp.tile([P, 512], f32, tag="nat")
                nc.sync.dma_start(
                    nat_b[:, 0:NB_FULL * R],
                    lb_d[e, 0:NB_FULL * P, :].rearrange("(vt v) r -> v vt r", v=P),
                )
                nc.sync.dma_start(
                    nat_b[0:REM, NB_FULL * R:NB_FULL * R + R],
                    lb_d[e, NB_FULL * P:VS, :],
                )
                for grp in range(8):
                    vts = [grp * 4 + j for j in range(4)]
                    ps = psp.tile([P, 512], f32, tag="bank")
                    off = 0
                    for vt in vts:
                        vsz = P if vt < NB_FULL else REM
                        nc.tensor.transpose(
                            ps[0:R, off:off + vsz],
                            nat_b[0:vsz, vt * R:(vt + 1) * R],
                            ident[0:vsz, 0:vsz],
                        )
                        off += vsz
                    st = btstp.tile([R, 512], f32r, tag="btst")
                    nc.vector.tensor_copy(st[:, 0:off], ps[0:R, 0:off])
                    v0 = vts[0] * P
                    nc.sync.dma_start(
                        bts_d[e * R:(e + 1) * R, v0:v0 + off], st[:, 0:off]
                    )

            def build_hT(t0, sl0, ntok):
                """Transpose hidden[t0:t0+ntok] into hT slots sl0:sl0+ntok."""
                for t8 in range(ntok // P):
                    for q in range(4):
                        nat_h = nathp.tile([P, 1024], f32, tag="nath")
                        nc.sync.dma_start(
                            nat_h,
                            hid_d[
                                t0 + t8 * P:t0 + (t8 + 1) * P,
                                q * 1024:(q + 1) * 1024,
                            ],
                        )
                        for ks in range(2):
                            ps = psp.tile([P, 512], f32, tag="bank")
                            for kk in range(4):
                                nc.tensor.transpose(
                                    ps[:, kk * P:(kk + 1) * P],
                                    nat_h[:, (ks * 4 + kk) * P:(ks * 4 + kk + 1) * P],
                                    ident,
                                )
                            k0 = q * 8 + ks * 4
                            nc.vector.tensor_copy(
                                hT_k[
                                    :, k0:k0 + 4,
                                    sl0 + t8 * P:sl0 + (t8 + 1) * P,
                                ],
                                ps,
                            )

            def build_mask(loads):
                """mask[p, slot] = (widx[token(slot)] == p//16), bf16 [128, TB].

                loads: list of (t0, sl0, n) idx segments to place at slots.
                """
                idxp = maskp.tile([1, TB], i32, tag="idxp")
                for (t0, sl0, n) in loads:
                    nc.sync.dma_start(idxp[:, sl0:sl0 + n], idx_d[:, t0:t0 + n])
                idx16 = maskp.tile([1, TB], bf16, tag="idx16")
                nc.vector.tensor_copy(idx16, idxp)
                nc.vector.tensor_scalar_mul(idx16, idx16, 16.0)
                bc = maskp.tile([P, TB], bf16, tag="bc")
                nc.gpsimd.partition_broadcast(bc, idx16)
                d = maskp.tile([P, TB], bf16, tag="d")
                nc.vector.tensor_scalar(d, bc, p_col, None, OP.subtract)
                u1 = maskp.tile([P, TB], bf16, tag="u1")
                nc.vector.tensor_scalar(u1, d, 0.0, None, OP.is_le)
                nc.vector.tensor_scalar(d, d, -15.0, None, OP.is_ge)
                mask = bc  # reuse slot: mask = u1 * d
                nc.vector.tensor_tensor(mask, u1, d, OP.mult)
                return mask

            def xa_windows(mxaT, mask, windows):
                """mxaT[:, w] = (A_all^T @ hT[:, :, w]) * mask[:, w] per window."""
                for (sl0, n) in windows:
                    xa_ps = psp.tile([P, 512], f32, tag="bank")
                    for kq in range(KT // 4):
                        atb = atldp.tile([P, 512], f32r, tag="atld")
                        nc.scalar.dma_start(
                            atb.rearrange("p (k e) -> p k e", e=P),
                            ats_d[4 * kq:4 * kq + 4, :, :].rearrange(
                                "k h e -> h k e"
                            ),
                        )
                        for kk in range(4):
                            k = 4 * kq + kk
                            nc.tensor.matmul(
                                xa_ps[:, 0:n],
                                atb[:, kk * P:(kk + 1) * P],
                                hT_k[:, k, sl0:sl0 + n],
                                start=(k == 0),
                                stop=(k == KT - 1),
                            )
                    nc.vector.tensor_tensor(
                        mxaT[:, sl0:sl0 + n],
                        xa_ps[:, 0:n],
                        mask[:, sl0:sl0 + n],
                        OP.mult,
                    )

            def finish_panel(accs, btk, mxaT, tiles, v0, np_):
                """lora matmul + copy-out + store for one panel."""
                for i, (tk, sl) in enumerate(tiles):
                    nc.tensor.matmul(
                        accs[i][:, 0:np_],
                        mxaT[:, sl:sl + P],
                        btk[:, 0:np_],
                        start=False,
                        stop=True,
                    )
                for i, (tk, sl) in enumerate(tiles):
                    o_sb = ostp.tile([P, 512], f32, tag="ost")
                    nc.vector.tensor_copy(o_sb[:, 0:np_], accs[i][:, 0:np_])
                    nc.sync.dma_start(
                        out_d[tk:tk + P, v0:v0 + np_], o_sb[:, 0:np_]
                    )

            def stream_panel(mxaT, tiles, v0, np_, namepfx):
                """One vocab panel streaming WTs; tiles = [(token, slot), ...]."""
                accs = [
                    psp.tile([P, 512], f32, tag="bank", name=f"{namepfx}_{v0}_{i}")
                    for i in range(len(tiles))
                ]
                btk = btldp.tile([P, 512], f32r, tag="btld")
                nc.scalar.dma_start(btk[:, 0:np_], bts_d[:, v0:v0 + np_])
                for kq in range(KT // 4):
                    wld = wldp.tile([P, 2048], f32r, tag="wld")
                    nc.scalar.dma_start(
                        wld.rearrange("p (k v) -> p k v", v=512)[:, :, 0:np_],
                        wts_d[4 * kq:4 * kq + 4, :, v0:v0 + np_].rearrange(
                            "k h v -> h k v"
                        ),
                    )
                    for kk in range(4):
                        k = 4 * kq + kk
                        for i, (tk, sl) in enumerate(tiles):
                            nc.tensor.matmul(
                                accs[i][:, 0:np_],
                                hT_k[:, k, sl:sl + P],
                                wld[:, kk * 512:kk * 512 + np_],
                                start=(k == 0),
                                stop=False,
                            )
                finish_panel(accs, btk, mxaT, tiles, v0, np_)

            # ================= sweep I: tokens 0:512, slots 0:512 =============
            build_hT(0, 0, 512)
            mask = build_mask([(0, 0, 512)])
            xa_windows(mxaT, mask, [(0, 512)])
            tiles_i = [(i * P, i * P) for i in range(4)]

            for pi, (v0, np_) in enumerate(PANELS):
                vbs = []
                off = 0
                while off < np_:
                    vsz = min(P, np_ - off)
                    vbs.append((off, vsz))
                    off += vsz
                accs = [
                    psp.tile([P, 512], f32, tag="bank", name=f"a1_{v0}_{i}")
                    for i in range(4)
                ]
                btk = btldp.tile([P, 512], f32r, tag="btld")
                nc.scalar.dma_start(btk[:, 0:np_], bts_d[:, v0:v0 + np_])
                for ks in range(8):
                    nats = []
                    for (vo, vsz) in vbs:
                        nat_w = natp.tile([P, 512], f32, tag="nat")
                        nc.sync.dma_start(
                            nat_w[0:vsz, :],
                            w_d[v0 + vo:v0 + vo + vsz, ks * 512:(ks + 1) * 512],
                        )
                        nats.append(nat_w)
                    for kk in range(4):
                        k = ks * 4 + kk
                        ps = psp.tile([P, 512], f32, tag="bank")
                        for (vo, vsz), nat_w in zip(vbs, nats):
                            nc.tensor.transpose(
                                ps[:, vo:vo + vsz],
                                nat_w[0:vsz, kk * P:(kk + 1) * P],
                                ident[0:vsz, 0:vsz],
                            )
                        wst = wstp.tile([P, 512], f32r, tag="wst")
                        nc.vector.tensor_copy(wst[:, 0:np_], ps[:, 0:np_])
                        nc.scalar.dma_start(
                            wts_d[k, :, v0:v0 + np_], wst[:, 0:np_]
                        )
                        for i in range(4):
                            nc.tensor.matmul(
                                accs[i][:, 0:np_],
                                hT_k[:, k, i * P:(i + 1) * P],
                                wst[:, 0:np_],
                                start=(k == 0),
                                stop=False,
                            )
                finish_panel(accs, btk, mxaT, tiles_i, v0, np_)
                if pi == 2:
                    # prebuild hT for tokens 512:1024 -> slots 512:1024
                    # (A1 only reads slots 0:512 — disjoint)
                    build_hT(512, 512, 512)

            # ============ sweep II: tokens 512:1280 ============
            # slots: 512:1024 (prebuilt) + 0:256 (built now, A1 done)
            build_hT(1024, 0, 256)
            mask = build_mask([(512, 512, 512), (1024, 0, 256)])
            xa_windows(mxaT, mask, [(512, 512), (0, 256)])
            tiles_ii = [(512 + i * P, (512 + i * P) % TB) for i in range(6)]
            for pi, (v0, np_) in enumerate(PANELS):
                stream_panel(mxaT, tiles_ii, v0, np_, "s2")
                if pi == 2:
                    # prebuild hT for tokens 1280:1536 -> slots 256:512
                    # (sweep II reads slots 512:1024 and 0:256 — disjoint)
                    build_hT(1280, 256, 256)

            # ============ sweep III: tokens 1280:2048 ============
            build_hT(1536, 512, 512)
            mask = build_mask([(1280, 256, 768)])
            xa_windows(mxaT, mask, [(256, 512), (768, 256)])
            tiles_iii = [(1280 + i * P, 256 + i * P) for i in range(6)]
            for (v0, np_) in PANELS:
                stream_panel(mxaT, tiles_iii, v0, np_, "s3")

        _free_mxa()
        free_hT()
        free_ident()

    nc.finalize()
    return nc


def _get_nc():
    if "nc" not in _CACHE:
        _CACHE["nc"] = _build_nc()
    return _CACHE["nc"]


def run_sharded(inputs, trace=False):
    from concourse import bass_utils

    hidden = np.ascontiguousarray(inputs["hidden_states"], dtype=np.float32)
    weight = np.ascontiguousarray(inputs["weight"], dtype=np.float32)
    lora_A = np.ascontiguousarray(inputs["lora_A"], dtype=np.float32).reshape(E * R, H)
    lora_B = np.ascontiguousarray(inputs["lora_B"], dtype=np.float32)
    widx = np.ascontiguousarray(inputs["weight_indices"], dtype=np.int32).reshape(1, T)

    nc = _get_nc()
    in_maps = []
    for c in range(NCORES):
        in_maps.append(
            {
                "hidden": hidden,
                "weight": weight[c * VS:(c + 1) * VS],
                "lora_a": lora_A,
                "lora_b": lora_B[:, c * VS:(c + 1) * VS, :],
                "widx": widx,
            }
        )
    res = bass_utils.run_bass_kernel_spmd(
        nc, in_maps, core_ids=list(range(NCORES)), trace=trace
    )
    out = np.concatenate([res.results[c]["out"] for c in range(NCORES)], axis=1)
    return out, res


def kernel(**inputs) -> np.ndarray:
    out, _ = run_sharded(inputs, trace=False)
    return out


# revision 10
# speedup vs baseline: 1.0173x; 1.0173x over previous
"""Trainium2 Bass kernel for ParallelLMHeadWithLoRA.

out[t, v] = hidden[t] @ weight[v]^T + xa[t] @ lora_B[e_t, v]^T
            where xa[t] = hidden[t] @ lora_A[e_t]^T,  e_t = weight_indices[t]

Sharding: column-parallel on vocab across 8 cores — weight and lora_B are
sharded along V (4000 rows/core), hidden / lora_A / weight_indices are
replicated.  Each core computes out[:, shard]; the host concatenates.

Per-core schedule (all fp32 data, fp32r matmuls).  hT is a 1024-token
ring of transposed-hidden slots; token sweeps map token tiles onto hT
slots mod 1024 so the next sweep's hT build overlaps the current sweep's
matmuls (disjoint slots):
  - consts, lora_A^T -> ATs, lora_B^T -> BTs (DRAM scratches)
  - sweep I   (tokens    0:512,  slots 0:512,   4 PSUM banks): FUSED —
    per vocab panel and k-tile, PE-transpose the natural weight block,
    matmul immediately, and store the transposed panel to WTs scratch.
  - sweep II  (tokens 512:1280,  slots 512:1024 + 0:256, 6 banks): WTs
  - sweep III (tokens 1280:2048, slots 256:1024, 6 banks): WTs
"""

import numpy as np

T = 2048
H = 4096
V = 32000
NCORES = 8
VS = V // NCORES  # 4000
E = 8
R = 16
P = 128
KT = H // P  # 32
TB = 1024  # hT slot-ring size (tokens)
PANELS = [(i * 512, 512) for i in range(7)] + [(3584, VS - 3584)]  # 7x512 + 416

_CACHE = {}


def _build_nc():
    from concourse import bacc
    import concourse.mybir as mybir
    from concourse.tile import TileContext
    from concourse.masks import make_identity

    f32 = mybir.dt.float32
    f32r = mybir.dt.float32r
    bf16 = mybir.dt.bfloat16
    i32 = mybir.dt.int32
    OP = mybir.AluOpType

    nc = bacc.Bacc("TRN2", target_bir_lowering=False, debug=False)

    hid_d = nc.dram_tensor("hidden", [T, H], f32, kind="ExternalInput")
    w_d = nc.dram_tensor("weight", [VS, H], f32, kind="ExternalInput")
    la_d = nc.dram_tensor("lora_a", [P, H], f32, kind="ExternalInput")  # [E*R, H]
    lb_d = nc.dram_tensor("lora_b", [E, VS, R], f32, kind="ExternalInput")
    idx_d = nc.dram_tensor("widx", [1, T], i32, kind="ExternalInput")
    out_d = nc.dram_tensor("out", [T, VS], f32, kind="ExternalOutput")

    wts_d = nc.dram_tensor("wts", [KT, P, VS], f32r, kind="Internal")
    ats_d = nc.dram_tensor("ats", [KT, P, P], f32r, kind="Internal")
    bts_d = nc.dram_tensor("bts", [P, VS], f32r, kind="Internal")

    with TileContext(nc) as tc:
        ident, free_ident = tc.tile([P, P], f32, name="ident")
        make_identity(nc, ident)
        hT, free_hT = tc.tile([P, KT * TB], f32r, name="hT")
        hT_k = hT.rearrange("p (k t) -> p k t", t=TB)
        mxaT, _free_mxa = tc.tile([P, TB], f32r, name="mxaT")

        with (
            tc.tile_pool(name="psp", bufs=8, space="PSUM") as psp,
            tc.tile_pool(name="natp", bufs=6) as natp,      # [128,512] f32 staging
            tc.tile_pool(name="wstp", bufs=3) as wstp,      # [128,512] f32r wT tiles
            tc.tile_pool(name="nathp", bufs=3) as nathp,    # [128,1024] f32 staging
            tc.tile_pool(name="wldp", bufs=2) as wldp,      # [128,2048] f32r loads
            tc.tile_pool(name="atldp", bufs=2) as atldp,    # [128,512] f32r loads
            tc.tile_pool(name="btldp", bufs=2) as btldp,    # [128,512] f32r loads
            tc.tile_pool(name="ostp", bufs=2) as ostp,      # [128,512] f32 out staging
            tc.tile_pool(name="maskp", bufs=1) as maskp,
            tc.tile_pool(name="btstp", bufs=1) as btstp,    # [16,512] f32r staging
        ):
            # ---- constants for mask ----
            p_col_i = maskp.tile([P, 1], i32, tag="pci")
            nc.gpsimd.iota(p_col_i, pattern=[[0, 1]], base=0, channel_multiplier=1)
            p_col = maskp.tile([P, 1], f32, tag="pcf")
            nc.vector.tensor_copy(p_col, p_col_i)

            # ---- lora_A^T -> ATs ----
            for q in range(4):
                nat_a = nathp.tile([P, 1024], f32, tag="nath")
                nc.sync.dma_start(nat_a, la_d[:, q * 1024:(q + 1) * 1024])
                for ks in range(2):
                    ps = psp.tile([P, 512], f32, tag="bank")
                    for kk in range(4):
                        nc.tensor.transpose(
                            ps[:, kk * P:(kk + 1) * P],
                            nat_a[:, (ks * 4 + kk) * P:(ks * 4 + kk + 1) * P],
                            ident,
                        )
                    st = wstp.tile([P, 512], f32r, tag="wst")
                    nc.vector.tensor_copy(st, ps)
                    k0 = q * 8 + ks * 4
                    nc.sync.dma_start(
                        ats_d[k0:k0 + 4, :, :].rearrange("k h e -> h k e"), st
                    )

            # ---- lora_B^T -> BTs ----
            NB_FULL = VS // P  # 31
            REM = VS % P       # 32
            for e in range(E):
                nat_b = natp.tile([P, 512], f32, tag="nat")
                nc.sync.dma_start(
                    nat_b[:, 0:NB_FULL * R],
                    lb_d[e, 0:NB_FULL * P, :].rearrange("(vt v) r -> v vt r", v=P),
                )
                nc.sync.dma_start(
                    nat_b[0:REM, NB_FULL * R:NB_FULL * R + R],
                    lb_d[e, NB_FULL * P:VS, :],
                )
                for grp in range(8):
                    vts = [grp * 4 + j for j in range(4)]
                    ps = psp.tile([P, 512], f32, tag="bank")
                    off = 0
                    for vt in vts:
                        vsz = P if vt < NB_FULL else REM
                        nc.tensor.transpose(
                            ps[0:R, off:off + vsz],
                            nat_b[0:vsz, vt * R:(vt + 1) * R],
                            ident[0:vsz, 0:vsz],
                        )
                        off += vsz
                    st = btstp.tile([R, 512], f32r, tag="btst")
                    nc.vector.tensor_copy(st[:, 0:off], ps[0:R, 0:off])
                    v0 = vts[0] * P
                    nc.sync.dma_start(
                        bts_d[e * R:(e + 1) * R, v0:v0 + off], st[:, 0:off]
                    )

            def build_hT(t0, sl0, ntok):
                """Transpose hidden[t0:t0+ntok] into hT slots sl0:sl0+ntok."""
                for t8 in range(ntok // P):
                    for q in range(4):
                        nat_h = nathp.tile([P, 1024], f32, tag="nath")
                        nc.sync.dma_start(
                            nat_h,
                            hid_d[
                                t0 + t8 * P:t0 + (t8 + 1) * P,
                                q * 1024:(q + 1) * 1024,
                            ],
                        )
                        for ks in range(2):
                            ps = psp.tile([P, 512], f32, tag="bank")
                            for kk in range(4):
                                nc.tensor.transpose(
                                    ps[:, kk * P:(kk + 1) * P],
                                    nat_h[:, (ks * 4 + kk) * P:(ks * 4 + kk + 1) * P],
                                    ident,
                                )
                            k0 = q * 8 + ks * 4
                            nc.vector.tensor_copy(
                                hT_k[
                                    :, k0:k0 + 4,
                                    sl0 + t8 * P:sl0 + (t8 + 1) * P,
                                ],
                                ps,
                            )

            def build_mask(loads):
                """mask[p, slot] = (widx[token(slot)] == p//16), bf16 [128, TB].

                loads: list of (t0, sl0, n) idx segments to place at slots.
                """
                idxp = maskp.tile([1, TB], i32, tag="idxp")
                for (t0, sl0, n) in loads:
                    nc.sync.dma_start(idxp[:, sl0:sl0 + n], idx_d[:, t0:t0 + n])
                idx16 = maskp.tile([1, TB], bf16, tag="idx16")
                nc.vector.tensor_copy(idx16, idxp)
                nc.vector.tensor_scalar_mul(idx16, idx16, 16.0)
                bc = maskp.tile([P, TB], bf16, tag="bc")
                nc.gpsimd.partition_broadcast(bc, idx16)
                d = maskp.tile([P, TB], bf16, tag="d")
                nc.vector.tensor_scalar(d, bc, p_col, None, OP.subtract)
                u1 = maskp.tile([P, TB], bf16, tag="u1")
                nc.vector.tensor_scalar(u1, d, 0.0, None, OP.is_le)
                nc.vector.tensor_scalar(d, d, -15.0, None, OP.is_ge)
                mask = bc  # reuse slot: mask = u1 * d
                nc.vector.tensor_tensor(mask, u1, d, OP.mult)
                return mask

            def xa_windows(mxaT, mask, windows):
                """mxaT[:, w] = (A_all^T @ hT[:, :, w]) * mask[:, w] per window."""
                for (sl0, n) in windows:
                    xa_ps = psp.tile([P, 512], f32, tag="bank")
                    for kq in range(KT // 4):
                        atb = atldp.tile([P, 512], f32r, tag="atld")
                        nc.scalar.dma_start(
                            atb.rearrange("p (k e) -> p k e", e=P),
                            ats_d[4 * kq:4 * kq + 4, :, :].rearrange(
                                "k h e -> h k e"
                            ),
                        )
                        for kk in range(4):
                            k = 4 * kq + kk
                            nc.tensor.matmul(
                                xa_ps[:, 0:n],
                                atb[:, kk * P:(kk + 1) * P],
                                hT_k[:, k, sl0:sl0 + n],
                                start=(k == 0),
                                stop=(k == KT - 1),
                            )
                    nc.vector.tensor_tensor(
                        mxaT[:, sl0:sl0 + n],
                        xa_ps[:, 0:n],
                        mask[:, sl0:sl0 + n],
                        OP.mult,
                    )

            def finish_panel(accs, btk, mxaT, tiles, v0, np_):
                """lora matmul + copy-out + store for one panel."""
                for i, (tk, sl) in enumerate(tiles):
                    nc.tensor.matmul(
                        accs[i][:, 0:np_],
                        mxaT[:, sl:sl + P],
                        btk[:, 0:np_],
                        start=False,
                        stop=True,
                    )
                for i, (tk, sl) in enumerate(tiles):
                    o_sb = ostp.tile([P, 512], f32, tag="ost")
                    nc.vector.tensor_copy(o_sb[:, 0:np_], accs[i][:, 0:np_])
                    nc.sync.dma_start(
                        out_d[tk:tk + P, v0:v0 + np_], o_sb[:, 0:np_]
                    )

            def stream_panel(mxaT, tiles, v0, np_, namepfx):
                """One vocab panel streaming WTs; tiles = [(token, slot), ...]."""
                accs = [
                    psp.tile([P, 512], f32, tag="bank", name=f"{namepfx}_{v0}_{i}")
                    for i in range(len(tiles))
                ]
                btk = btldp.tile([P, 512], f32r, tag="btld")
                nc.scalar.dma_start(btk[:, 0:np_], bts_d[:, v0:v0 + np_])
                for kq in range(KT // 4):
                    wld = wldp.tile([P, 2048], f32r, tag="wld")
                    nc.scalar.dma_start(
                        wld.rearrange("p (k v) -> p k v", v=512)[:, :, 0:np_],
                        wts_d[4 * kq:4 * kq + 4, :, v0:v0 + np_].rearrange(
                            "k h v -> h k v"
                        ),
                    )
                    for kk in range(4):
                        k = 4 * kq + kk
                        for i, (tk, sl) in enumerate(tiles):
                            nc.tensor.matmul(
                                accs[i][:, 0:np_],
                                hT_k[:, k, sl:sl + P],
                                wld[:, kk * 512:kk * 512 + np_],
                                start=(k == 0),
                                stop=False,
                            )
                finish_panel(accs, btk, mxaT, tiles, v0, np_)

            # ================= sweep I: tokens 0:768, slots 0:768 ============
            build_hT(0, 0, 768)
            mask = build_mask([(0, 0, 768)])
            xa_windows(mxaT, mask, [(0, 512), (512, 256)])
            tiles_i = [(i * P, i * P) for i in range(6)]

            for pi, (v0, np_) in enumerate(PANELS):
                vbs = []
                off = 0
                while off < np_:
                    vsz = min(P, np_ - off)
                    vbs.append((off, vsz))
                    off += vsz
                accs = [
                    psp.tile([P, 512], f32, tag="bank", name=f"a1_{v0}_{i}")
                    for i in range(6)
                ]
                btk = btldp.tile([P, 512], f32r, tag="btld")
                nc.scalar.dma_start(btk[:, 0:np_], bts_d[:, v0:v0 + np_])
                for ks in range(8):
                    nats = []
                    for (vo, vsz) in vbs:
                        nat_w = natp.tile([P, 512], f32, tag="nat")
                        nc.sync.dma_start(
                            nat_w[0:vsz, :],
                            w_d[v0 + vo:v0 + vo + vsz, ks * 512:(ks + 1) * 512],
                        )
                        nats.append(nat_w)
                    for kk in range(4):
                        k = ks * 4 + kk
                        ps = psp.tile([P, 512], f32, tag="bank")
                        for (vo, vsz), nat_w in zip(vbs, nats):
                            nc.tensor.transpose(
                                ps[:, vo:vo + vsz],
                                nat_w[0:vsz, kk * P:(kk + 1) * P],
                                ident[0:vsz, 0:vsz],
                            )
                        wst = wstp.tile([P, 512], f32r, tag="wst")
                        nc.vector.tensor_copy(wst[:, 0:np_], ps[:, 0:np_])
                        nc.scalar.dma_start(
                            wts_d[k, :, v0:v0 + np_], wst[:, 0:np_]
                        )
                        for i in range(6):
                            nc.tensor.matmul(
                                accs[i][:, 0:np_],
                                hT_k[:, k, i * P:(i + 1) * P],
                                wst[:, 0:np_],
                                start=(k == 0),
                                stop=False,
                            )
                finish_panel(accs, btk, mxaT, tiles_i, v0, np_)
                if pi == 2:
                    # prebuild hT for tokens 768:1024 -> slots 768:1024
                    build_hT(768, 768, 256)

            # ============ sweep II: tokens 768:1408, 5 tiles ============
            build_hT(1024, 0, 384)
            mask = build_mask([(768, 768, 256), (1024, 0, 384)])
            xa_windows(mxaT, mask, [(768, 256), (0, 384)])
            tiles_ii = [(768 + i * P, (768 + i * P) % TB) for i in range(5)]
            for pi, (v0, np_) in enumerate(PANELS):
                stream_panel(mxaT, tiles_ii, v0, np_, "s2")
                if pi == 2:
                    # prebuild hT for tokens 1408:1792 -> slots 384:768
                    build_hT(1408, 384, 384)

            # ============ sweep III: tokens 1408:2048, 5 tiles ============
            build_hT(1792, 768, 256)
            mask = build_mask([(1408, 384, 384), (1792, 768, 256)])
            xa_windows(mxaT, mask, [(384, 384), (768, 256)])
            tiles_iii = [(1408 + i * P, 384 + i * P) for i in range(5)]
            for (v0, np_) in PANELS:
                stream_panel(mxaT, tiles_iii, v0, np_, "s3")

        _free_mxa()
        free_hT()
        free_ident()

    nc.finalize()
    return nc


def _get_nc():
    if "nc" not in _CACHE:
        _CACHE["nc"] = _build_nc()
    return _CACHE["nc"]


def run_sharded(inputs, trace=False):
    from concourse import bass_utils

    hidden = np.ascontiguousarray(inputs["hidden_states"], dtype=np.float32)
    weight = np.ascontiguousarray(inputs["weight"], dtype=np.float32)
    lora_A = np.ascontiguousarray(inputs["lora_A"], dtype=np.float32).reshape(E * R, H)
    lora_B = np.ascontiguousarray(inputs["lora_B"], dtype=np.float32)
    widx = np.ascontiguousarray(inputs["weight_indices"], dtype=np.int32).reshape(1, T)

    nc = _get_nc()
    in_maps = []
    for c in range(NCORES):
        in_maps.append(
            {
                "hidden": hidden,
                "weight": weight[c * VS:(c + 1) * VS],
                "lora_a": lora_A,
                "lora_b": lora_B[:, c * VS:(c + 1) * VS, :],
                "widx": widx,
            }
        )
    res = bass_utils.run_bass_kernel_spmd(
        nc, in_maps, core_ids=list(range(NCORES)), trace=trace
    )
    out = np.concatenate([res.results[c]["out"] for c in range(NCORES)], axis=1)
    return out, res


def kernel(**inputs) -> np.ndarray:
    out, _ = run_sharded(inputs, trace=False)
    return out


# revision 11
# speedup vs baseline: 1.0181x; 1.0007x over previous
"""Trainium2 Bass kernel for ParallelLMHeadWithLoRA.

out[t, v] = hidden[t] @ weight[v]^T + xa[t] @ lora_B[e_t, v]^T
            where xa[t] = hidden[t] @ lora_A[e_t]^T,  e_t = weight_indices[t]

Sharding: column-parallel on vocab across 8 cores — weight and lora_B are
sharded along V (4000 rows/core), hidden / lora_A / weight_indices are
replicated.  Each core computes out[:, shard]; the host concatenates.

Per-core schedule (all fp32 data, fp32r matmuls).  hT is a 1024-token
ring of transposed-hidden slots; token sweeps map token tiles onto hT
slots mod 1024 so the next sweep's hT build overlaps the current sweep's
matmuls (disjoint slots):
  - consts, lora_A^T -> ATs, lora_B^T -> BTs (DRAM scratches)
  - sweep I   (tokens    0:512,  slots 0:512,   4 PSUM banks): FUSED —
    per vocab panel and k-tile, PE-transpose the natural weight block,
    matmul immediately, and store the transposed panel to WTs scratch.
  - sweep II  (tokens 512:1280,  slots 512:1024 + 0:256, 6 banks): WTs
  - sweep III (tokens 1280:2048, slots 256:1024, 6 banks): WTs
"""

import numpy as np

T = 2048
H = 4096
V = 32000
NCORES = 8
VS = V // NCORES  # 4000
E = 8
R = 16
P = 128
KT = H // P  # 32
TB = 1024  # hT slot-ring size (tokens)
PANELS = [(i * 512, 512) for i in range(7)] + [(3584, VS - 3584)]  # 7x512 + 416

_CACHE = {}


def _build_nc():
    from concourse import bacc
    import concourse.mybir as mybir
    from concourse.tile import TileContext
    from concourse.masks import make_identity

    f32 = mybir.dt.float32
    f32r = mybir.dt.float32r
    bf16 = mybir.dt.bfloat16
    i32 = mybir.dt.int32
    OP = mybir.AluOpType

    nc = bacc.Bacc("TRN2", target_bir_lowering=False, debug=False)

    hid_d = nc.dram_tensor("hidden", [T, H], f32, kind="ExternalInput")
    w_d = nc.dram_tensor("weight", [VS, H], f32, kind="ExternalInput")
    la_d = nc.dram_tensor("lora_a", [P, H], f32, kind="ExternalInput")  # [E*R, H]
    lb_d = nc.dram_tensor("lora_b", [E, VS, R], f32, kind="ExternalInput")
    idx_d = nc.dram_tensor("widx", [1, T], i32, kind="ExternalInput")
    out_d = nc.dram_tensor("out", [T, VS], f32, kind="ExternalOutput")

    wts_d = nc.dram_tensor("wts", [KT, P, VS], f32r, kind="Internal")
    ats_d = nc.dram_tensor("ats", [KT, P, P], f32r, kind="Internal")
    bts_d = nc.dram_tensor("bts", [P, VS], f32r, kind="Internal")

    with TileContext(nc) as tc:
        ident, free_ident = tc.tile([P, P], f32, name="ident")
        make_identity(nc, ident)
        hT, free_hT = tc.tile([P, KT * TB], f32r, name="hT")
        hT_k = hT.rearrange("p (k t) -> p k t", t=TB)
        mxaT, _free_mxa = tc.tile([P, TB], f32r, name="mxaT")

        with (
            tc.tile_pool(name="psp", bufs=8, space="PSUM") as psp,
            tc.tile_pool(name="natp", bufs=6) as natp,      # [128,512] f32 staging
            tc.tile_pool(name="wstp", bufs=3) as wstp,      # [128,512] f32r wT tiles
            tc.tile_pool(name="nathp", bufs=2) as nathp,    # [128,1024] f32 staging
            tc.tile_pool(name="wldp", bufs=3) as wldp,      # [128,2048] f32r loads
            tc.tile_pool(name="atldp", bufs=1) as atldp,    # [128,512] f32r loads
            tc.tile_pool(name="btldp", bufs=2) as btldp,    # [128,512] f32r loads
            tc.tile_pool(name="ostp", bufs=2) as ostp,      # [128,512] f32 out staging
            tc.tile_pool(name="maskp", bufs=1) as maskp,
            tc.tile_pool(name="btstp", bufs=1) as btstp,    # [16,512] f32r staging
        ):
            # ---- constants for mask ----
            p_col_i = maskp.tile([P, 1], i32, tag="pci")
            nc.gpsimd.iota(p_col_i, pattern=[[0, 1]], base=0, channel_multiplier=1)
            p_col = maskp.tile([P, 1], f32, tag="pcf")
            nc.vector.tensor_copy(p_col, p_col_i)

            # ---- lora_A^T -> ATs ----
            for q in range(4):
                nat_a = nathp.tile([P, 1024], f32, tag="nath")
                nc.sync.dma_start(nat_a, la_d[:, q * 1024:(q + 1) * 1024])
                for ks in range(2):
                    ps = psp.tile([P, 512], f32, tag="bank")
                    for kk in range(4):
                        nc.tensor.transpose(
                            ps[:, kk * P:(kk + 1) * P],
                            nat_a[:, (ks * 4 + kk) * P:(ks * 4 + kk + 1) * P],
                            ident,
                        )
                    st = wstp.tile([P, 512], f32r, tag="wst")
                    nc.vector.tensor_copy(st, ps)
                    k0 = q * 8 + ks * 4
                    nc.sync.dma_start(
                        ats_d[k0:k0 + 4, :, :].rearrange("k h e -> h k e"), st
                    )

            # ---- lora_B^T -> BTs ----
            NB_FULL = VS // P  # 31
            REM = VS % P       # 32
            for e in range(E):
                nat_b = natp.tile([P, 512], f32, tag="nat")
                nc.sync.dma_start(
                    nat_b[:, 0:NB_FULL * R],
                    lb_d[e, 0:NB_FULL * P, :].rearrange("(vt v) r -> v vt r", v=P),
                )
                nc.sync.dma_start(
                    nat_b[0:REM, NB_FULL * R:NB_FULL * R + R],
                    lb_d[e, NB_FULL * P:VS, :],
                )
                for grp in range(8):
                    vts = [grp * 4 + j for j in range(4)]
                    ps = psp.tile([P, 512], f32, tag="bank")
                    off = 0
                    for vt in vts:
                        vsz = P if vt < NB_FULL else REM
                        nc.tensor.transpose(
                            ps[0:R, off:off + vsz],
                            nat_b[0:vsz, vt * R:(vt + 1) * R],
                            ident[0:vsz, 0:vsz],
                        )
                        off += vsz
                    st = btstp.tile([R, 512], f32r, tag="btst")
                    nc.vector.tensor_copy(st[:, 0:off], ps[0:R, 0:off])
                    v0 = vts[0] * P
                    nc.sync.dma_start(
                        bts_d[e * R:(e + 1) * R, v0:v0 + off], st[:, 0:off]
                    )

            def build_hT(t0, sl0, ntok):
                """Transpose hidden[t0:t0+ntok] into hT slots sl0:sl0+ntok."""
                for t8 in range(ntok // P):
                    for q in range(4):
                        nat_h = nathp.tile([P, 1024], f32, tag="nath")
                        nc.sync.dma_start(
                            nat_h,
                            hid_d[
                                t0 + t8 * P:t0 + (t8 + 1) * P,
                                q * 1024:(q + 1) * 1024,
                            ],
                        )
                        for ks in range(2):
                            ps = psp.tile([P, 512], f32, tag="bank")
                            for kk in range(4):
                                nc.tensor.transpose(
                                    ps[:, kk * P:(kk + 1) * P],
                                    nat_h[:, (ks * 4 + kk) * P:(ks * 4 + kk + 1) * P],
                                    ident,
                                )
                            k0 = q * 8 + ks * 4
                            nc.vector.tensor_copy(
                                hT_k[
                                    :, k0:k0 + 4,
                                    sl0 + t8 * P:sl0 + (t8 + 1) * P,
                                ],
                                ps,
                            )

            def xa_window(t0, sl0, n):
                """mask+xa for one token window (<=512 wide, one idx segment):
                mxaT[:, sl0:sl0+n] = (A_all^T @ hT[:, :, sl0:sl0+n]) * mask."""
                idxp = maskp.tile([1, 512], i32, tag="idxp")
                nc.sync.dma_start(idxp[:, 0:n], idx_d[:, t0:t0 + n])
                idx16 = maskp.tile([1, 512], bf16, tag="idx16")
                nc.vector.tensor_copy(idx16[:, 0:n], idxp[:, 0:n])
                nc.vector.tensor_scalar_mul(idx16[:, 0:n], idx16[:, 0:n], 16.0)
                bc = maskp.tile([P, 512], bf16, tag="bc")
                nc.gpsimd.partition_broadcast(bc[:, 0:n], idx16[:, 0:n])
                d = maskp.tile([P, 512], bf16, tag="d")
                nc.vector.tensor_scalar(
                    d[:, 0:n], bc[:, 0:n], p_col, None, OP.subtract
                )
                u1 = maskp.tile([P, 512], bf16, tag="u1")
                nc.vector.tensor_scalar(u1[:, 0:n], d[:, 0:n], 0.0, None, OP.is_le)
                nc.vector.tensor_scalar(d[:, 0:n], d[:, 0:n], -15.0, None, OP.is_ge)
                nc.vector.tensor_tensor(
                    bc[:, 0:n], u1[:, 0:n], d[:, 0:n], OP.mult
                )
                xa_ps = psp.tile([P, 512], f32, tag="bank")
                for kq in range(KT // 4):
                    atb = atldp.tile([P, 512], f32r, tag="atld")
                    nc.scalar.dma_start(
                        atb.rearrange("p (k e) -> p k e", e=P),
                        ats_d[4 * kq:4 * kq + 4, :, :].rearrange("k h e -> h k e"),
                    )
                    for kk in range(4):
                        k = 4 * kq + kk
                        nc.tensor.matmul(
                            xa_ps[:, 0:n],
                            atb[:, kk * P:(kk + 1) * P],
                            hT_k[:, k, sl0:sl0 + n],
                            start=(k == 0),
                            stop=(k == KT - 1),
                        )
                nc.vector.tensor_tensor(
                    mxaT[:, sl0:sl0 + n], xa_ps[:, 0:n], bc[:, 0:n], OP.mult
                )

            def finish_panel(accs, btk, mxaT, tiles, v0, np_):
                """lora matmul + copy-out + store for one panel."""
                for i, (tk, sl) in enumerate(tiles):
                    nc.tensor.matmul(
                        accs[i][:, 0:np_],
                        mxaT[:, sl:sl + P],
                        btk[:, 0:np_],
                        start=False,
                        stop=True,
                    )
                for i, (tk, sl) in enumerate(tiles):
                    o_sb = ostp.tile([P, 512], f32, tag="ost")
                    nc.vector.tensor_copy(o_sb[:, 0:np_], accs[i][:, 0:np_])
                    nc.sync.dma_start(
                        out_d[tk:tk + P, v0:v0 + np_], o_sb[:, 0:np_]
                    )

            def stream_panel(mxaT, tiles, v0, np_, namepfx):
                """One vocab panel streaming WTs; tiles = [(token, slot), ...]."""
                accs = [
                    psp.tile([P, 512], f32, tag="bank", name=f"{namepfx}_{v0}_{i}")
                    for i in range(len(tiles))
                ]
                btk = btldp.tile([P, 512], f32r, tag="btld")
                nc.scalar.dma_start(btk[:, 0:np_], bts_d[:, v0:v0 + np_])
                for kq in range(KT // 4):
                    wld = wldp.tile([P, 2048], f32r, tag="wld")
                    nc.scalar.dma_start(
                        wld.rearrange("p (k v) -> p k v", v=512)[:, :, 0:np_],
                        wts_d[4 * kq:4 * kq + 4, :, v0:v0 + np_].rearrange(
                            "k h v -> h k v"
                        ),
                    )
                    for kk in range(4):
                        k = 4 * kq + kk
                        for i, (tk, sl) in enumerate(tiles):
                            nc.tensor.matmul(
                                accs[i][:, 0:np_],
                                hT_k[:, k, sl:sl + P],
                                wld[:, kk * 512:kk * 512 + np_],
                                start=(k == 0),
                                stop=False,
                            )
                finish_panel(accs, btk, mxaT, tiles, v0, np_)

            # ================= sweep I: tokens 0:768, slots 0:768 ============
            build_hT(0, 0, 768)
            xa_window(0, 0, 512)
            xa_window(512, 512, 256)
            tiles_i = [(i * P, i * P) for i in range(6)]

            for pi, (v0, np_) in enumerate(PANELS):
                vbs = []
                off = 0
                while off < np_:
                    vsz = min(P, np_ - off)
                    vbs.append((off, vsz))
                    off += vsz
                accs = [
                    psp.tile([P, 512], f32, tag="bank", name=f"a1_{v0}_{i}")
                    for i in range(6)
                ]
                btk = btldp.tile([P, 512], f32r, tag="btld")
                nc.scalar.dma_start(btk[:, 0:np_], bts_d[:, v0:v0 + np_])
                for ks in range(8):
                    nats = []
                    for (vo, vsz) in vbs:
                        nat_w = natp.tile([P, 512], f32, tag="nat")
                        nc.sync.dma_start(
                            nat_w[0:vsz, :],
                            w_d[v0 + vo:v0 + vo + vsz, ks * 512:(ks + 1) * 512],
                        )
                        nats.append(nat_w)
                    for kk in range(4):
                        k = ks * 4 + kk
                        ps = psp.tile([P, 512], f32, tag="bank")
                        for (vo, vsz), nat_w in zip(vbs, nats):
                            nc.tensor.transpose(
                                ps[:, vo:vo + vsz],
                                nat_w[0:vsz, kk * P:(kk + 1) * P],
                                ident[0:vsz, 0:vsz],
                            )
                        wst = wstp.tile([P, 512], f32r, tag="wst")
                        nc.vector.tensor_copy(wst[:, 0:np_], ps[:, 0:np_])
                        nc.scalar.dma_start(
                            wts_d[k, :, v0:v0 + np_], wst[:, 0:np_]
                        )
                        for i in range(6):
                            nc.tensor.matmul(
                                accs[i][:, 0:np_],
                                hT_k[:, k, i * P:(i + 1) * P],
                                wst[:, 0:np_],
                                start=(k == 0),
                                stop=False,
                            )
                finish_panel(accs, btk, mxaT, tiles_i, v0, np_)
                if pi == 2:
                    # prebuild hT for tokens 768:1024 -> slots 768:1024
                    build_hT(768, 768, 256)

            # ============ sweep II: tokens 768:1408, 5 tiles ============
            build_hT(1024, 0, 384)
            xa_window(768, 768, 256)
            xa_window(1024, 0, 384)
            tiles_ii = [(768 + i * P, (768 + i * P) % TB) for i in range(5)]
            for pi, (v0, np_) in enumerate(PANELS):
                stream_panel(mxaT, tiles_ii, v0, np_, "s2")
                if pi == 2:
                    # prebuild hT for tokens 1408:1792 -> slots 384:768
                    build_hT(1408, 384, 384)

            # ============ sweep III: tokens 1408:2048, 5 tiles ============
            build_hT(1792, 768, 256)
            xa_window(1408, 384, 384)
            xa_window(1792, 768, 256)
            tiles_iii = [(1408 + i * P, 384 + i * P) for i in range(5)]
            for (v0, np_) in PANELS:
                stream_panel(mxaT, tiles_iii, v0, np_, "s3")

        _free_mxa()
        free_hT()
        free_ident()

    nc.finalize()
    return nc


def _get_nc():
    if "nc" not in _CACHE:
        _CACHE["nc"] = _build_nc()
    return _CACHE["nc"]


def run_sharded(inputs, trace=False):
    from concourse import bass_utils

    hidden = np.ascontiguousarray(inputs["hidden_states"], dtype=np.float32)
    weight = np.ascontiguousarray(inputs["weight"], dtype=np.float32)
    lora_A = np.ascontiguousarray(inputs["lora_A"], dtype=np.float32).reshape(E * R, H)
    lora_B = np.ascontiguousarray(inputs["lora_B"], dtype=np.float32)
    widx = np.ascontiguousarray(inputs["weight_indices"], dtype=np.int32).reshape(1, T)

    nc = _get_nc()
    in_maps = []
    for c in range(NCORES):
        in_maps.append(
            {
                "hidden": hidden,
                "weight": weight[c * VS:(c + 1) * VS],
                "lora_a": lora_A,
                "lora_b": lora_B[:, c * VS:(c + 1) * VS, :],
                "widx": widx,
            }
        )
    res = bass_utils.run_bass_kernel_spmd(
        nc, in_maps, core_ids=list(range(NCORES)), trace=trace
    )
    out = np.concatenate([res.results[c]["out"] for c in range(NCORES)], axis=1)
    return out, res


def kernel(**inputs) -> np.ndarray:
    out, _ = run_sharded(inputs, trace=False)
    return out
